# revision 6
# baseline (speedup 1.0000x reference)
"""Trainium2 Bass kernel for nn_DelayLIFSNN.

Architecture (per reference):
  x (B, T0, J) -> delay_conv(w0,p0) -> BN(global batch stats) -> LIF
               -> delay_conv(w1,p1) -> BN -> LIF
               -> delay_conv(wr,pr) -> LI readout -> sum_t softmax_o -> (B, O)

Sharding: data-parallel over batch B across 8 cores (B_loc=32/core);
weights replicated; BN stats all-reduced ((128, 2*HT) f32 = 4KB each).

Wire format: raw transposed weights (w.T, p.T) — the Dcls gaussian-
interpolated delay kernels (K=25 taps) are expanded ON DEVICE
(DVE/ACT: d = p-(k-12); e = exp(-2 d^2); normalize over k; * w),
cutting host->device traffic from ~40MB/core to ~9MB/core.

Matmul precision strategy (spike flips make the output chaotic in the
conv operand precision; measured: tf32-level quantization -> 3.9e-2 rel
err vs the 2e-2 gate, so fp32r is unusable):
  conv1 (x * w0): plain fp32 matmuls (4 cyc/row).
  conv2 (spikes * w1): weights split hi+lo bf16 (2 matmuls, 1 cyc/row
    each); spikes are 0/1 = EXACT in bf16. Combined weight error ~1e-5.
  conv3 (spikes * wr): bf16-hi only (readout has no threshold
    nonlinearity; measured rel err impact nil).

LIF: DVE runs the 2-op recurrence (ut = beta*U + scn; U = (ut<1)*ut);
Pool (gpsimd) computes spikes (ut>=1 -> bf16) and the BN-affine scn
precompute in parallel.

LI readout + softmax tail via PE: u[t,(b,o)] = sum_t' L[t',t] y3[t',b,o]
with L = beta^(t-t') lower-tri Toeplitz (wire input), then exp (no max
subtraction needed: |u| < 20), per-(t,b) normalize, and a ones-matmul
partition-reduce over t. Output [1, B*O].

Activation layouts:
  conv rhs:   [ch_part<=128, t*B + b]  (DRAM src: [C, Tpad, B])
  conv psum:  [out_part 128, t*B + b] per (ht, time-tile)
  y DRAM:     [C, T, B] f32 ; spikes DRAM: [C, T+PADT, B] bf16
  LIF tiles:  [128, t*(HT*B) + ht*B + b]
  y3t DRAM:   [T3, B, O] f32 (scatter-stored by conv3)
"""

import sys
import numpy as np

try:
    import concourse.bass as bass
except ImportError:  # grading env fallback
    sys.path.insert(0, "/opt/trn_rl_repo")
    import concourse.bass as bass

import concourse.mybir as mybir
import concourse.tile as tile
from contextlib import ExitStack
from concourse import bacc
from concourse.bass_utils import run_bass_kernel_spmd

F32 = mybir.dt.float32
BF16 = mybir.dt.bfloat16
AF = mybir.ActivationFunctionType
OP = mybir.AluOpType


class Cfg:
    def __init__(self, T0=300, B_loc=32, J=140, H=512, O=20, K=25, n_cores=8,
                 BETA=0.95, THRESH=1.0, SIG=0.5, EPS=1e-5, NT=16, CH=48,
                 CT1=5, CT2=6, dbg=False):
        self.T0, self.B_loc, self.J, self.H, self.O, self.K = T0, B_loc, J, H, O, K
        self.n_cores = n_cores
        self.BETA, self.THRESH, self.SIG, self.EPS = BETA, THRESH, SIG, EPS
        self.LPAD, self.RPAD = K - 1, (K - 1) // 2
        self.PADT = self.LPAD + self.RPAD                      # 36
        self.T1 = T0 + self.RPAD                               # 312
        self.T2 = self.T1 + self.RPAD                          # 324
        self.T3 = self.T2 + self.RPAD                          # 336
        self.NT = NT                                           # out-steps per matmul tile
        self.CH = CH                                           # LIF chunk steps
        self.CT1 = CT1                                         # conv1 time-tiles per chunk
        self.CT2 = CT2                                         # conv2/3 time-tiles per chunk
        self.HT = (H + 127) // 128                             # h tiles (4)
        self.B_tot = B_loc * n_cores
        self.dbg = dbg


def split_tiles(total, size):
    out = []
    t = 0
    while t < total:
        n = min(size, total - t)
        out.append((t, n))
        t += n
    return out


def bc(ap, axis, count):
    """Insert a stride-0 (broadcast) axis at position `axis` of an AP."""
    dims = [list(d) for d in ap.ap]
    dims.insert(axis, [0, count])
    return bass.AP(tensor=ap.tensor, offset=ap.offset, ap=dims)


def build_kernel(cfg: Cfg):
    c = cfg
    B, HT, K, H, O, J = c.B_loc, c.HT, c.K, c.H, c.O, c.J
    nc = bacc.Bacc("TRN2", target_bir_lowering=False, debug=False,
                   num_devices=c.n_cores)

    tts1 = split_tiles(c.T1, c.NT)
    tts2 = split_tiles(c.T2, c.NT)
    tts3 = split_tiles(c.T3, c.NT)
    n1slots = len(tts1)
    n2slots = len(tts2)
    cts_J = split_tiles(J, 128)
    cts_H = split_tiles(H, 128)

    # ---- I/O ----
    xp = nc.dram_tensor("xp", [J, c.T0 + c.PADT, B], F32, kind="ExternalInput")
    w0r = nc.dram_tensor("w0r", [J, H], F32, kind="ExternalInput")
    p0r = nc.dram_tensor("p0r", [J, H], F32, kind="ExternalInput")
    w1r = nc.dram_tensor("w1r", [H, H], F32, kind="ExternalInput")
    p1r = nc.dram_tensor("p1r", [H, H], F32, kind="ExternalInput")
    wrr = nc.dram_tensor("wrr", [H, O], F32, kind="ExternalInput")
    prr = nc.dram_tensor("prr", [H, O], F32, kind="ExternalInput")
    Lm = nc.dram_tensor("Lm", [c.T3, c.T3], F32, kind="ExternalInput")
    g0m = nc.dram_tensor("g0m", [128, HT], F32, kind="ExternalInput")
    b0m = nc.dram_tensor("b0m", [128, HT], F32, kind="ExternalInput")
    g1m = nc.dram_tensor("g1m", [128, HT], F32, kind="ExternalInput")
    b1m = nc.dram_tensor("b1m", [128, HT], F32, kind="ExternalInput")
    out = nc.dram_tensor("out", [1, B * O], F32, kind="ExternalOutput")
    if c.dbg:
        d_y1 = nc.dram_tensor("d_y1", [H, c.T1, B], F32, kind="ExternalOutput")
        d_s1 = nc.dram_tensor("d_s1", [H, c.T1 + c.PADT, B], BF16,
                              kind="ExternalOutput")
        d_y2 = nc.dram_tensor("d_y2", [H, c.T2, B], F32, kind="ExternalOutput")
        d_y3 = nc.dram_tensor("d_y3", [c.T3, B, O], F32, kind="ExternalOutput")
        d_w0 = nc.dram_tensor("d_w0", [K, J, H], F32, kind="ExternalOutput")
        d_w1 = nc.dram_tensor("d_w1", [2, K, H, H], BF16, kind="ExternalOutput")

    with tile.TileContext(nc) as tc, ExitStack() as ctx:
        dram = ctx.enter_context(tc.tile_pool(name="dram", bufs=1, space="DRAM"))
        w0x = dram.tile([K, J, H], F32, name="w0x")
        w1x = dram.tile([2, K, H, H], BF16, name="w1x")
        wrx = dram.tile([K, H, O], BF16, name="wrx")
        y1d = dram.tile([H, c.T1, B], F32, name="y1d")
        s1d = dram.tile([H, c.T1 + c.PADT, B], BF16, name="s1d")
        y2d = dram.tile([H, c.T2, B], F32, name="y2d")
        s2d = dram.tile([H, c.T2 + c.PADT, B], BF16, name="s2d")
        y3t = dram.tile([c.T3, B, O], F32, name="y3t")
        cc_space = "Shared" if c.n_cores > 4 else "Local"
        cc1i = dram.tile([128, 2 * HT], F32, name="cc1i")
        cc1o = dram.tile([128, 2 * HT], F32, name="cc1o", addr_space=cc_space)
        cc2i = dram.tile([128, 2 * HT], F32, name="cc2i")
        cc2o = dram.tile([128, 2 * HT], F32, name="cc2o", addr_space=cc_space)

        glob = ctx.enter_context(tc.tile_pool(name="glob", bufs=1))

        # persistent small tiles
        sum1 = glob.tile([128, HT * n1slots], F32, name="sum1")
        sq1 = glob.tile([128, HT * n1slots], F32, name="sq1")
        sum2 = glob.tile([128, HT * n2slots], F32, name="sum2")
        sq2 = glob.tile([128, HT * n2slots], F32, name="sq2")
        gam0 = glob.tile([128, HT], F32, name="gam0")
        bet0 = glob.tile([128, HT], F32, name="bet0")
        gam1 = glob.tile([128, HT], F32, name="gam1")
        bet1 = glob.tile([128, HT], F32, name="bet1")
        nc.sync.dma_start(out=gam0, in_=g0m.ap())
        nc.sync.dma_start(out=bet0, in_=b0m.ap())
        nc.sync.dma_start(out=gam1, in_=g1m.ap())
        nc.sync.dma_start(out=bet1, in_=b1m.ap())
        A1 = glob.tile([128, HT], F32, name="A1")
        C1b = glob.tile([128, HT * B], F32, name="C1b")
        A2 = glob.tile([128, HT], F32, name="A2")
        C2b = glob.tile([128, HT * B], F32, name="C2b")
        zpad = glob.tile([128, c.LPAD * B], BF16, name="zpad")
        nc.vector.memset(zpad, 0.0)
        negth = glob.tile([128, 1], F32, name="negth")
        nc.vector.memset(negth, float(-cfg.THRESH))

        # zero the pad regions of the spike dram buffers
        for sd, T in ((s1d, c.T1), (s2d, c.T2)):
            for ct, (h0, pw) in enumerate(cts_H):
                nc.sync.dma_start(
                    out=sd[h0:h0 + pw, 0:c.LPAD, :],
                    in_=zpad.rearrange("p (t b) -> p t b", b=B)[:pw])
                nc.sync.dma_start(
                    out=sd[h0:h0 + pw, T + c.LPAD:T + c.PADT, :],
                    in_=zpad.rearrange("p (t b) -> p t b", b=B)[:pw, :c.RPAD, :])

        # =============== weight expansion (device-side Dcls gauss) ========
        exp_pools = ExitStack()
        ep = exp_pools.enter_context(tc.tile_pool(name="exp", bufs=1))
        et = exp_pools.enter_context(tc.tile_pool(name="expt", bufs=2))

        def expand_ct(wr_, pr_, c0, pw, M, dst, split, tagp):
            """Expand one Cin tile: dst[k, c0:c0+pw, :] = w*gauss_norm(k,p).

            Processes M in chunks of <=256 columns to bound SBUF.
            split=None: dst is f32 [K, C, M].
            split=2: dst is bf16 [2, K, C, M] (hi, lo planes).
            split=1: dst is bf16 [K, C, M] (hi only).
            """
            for (m0, mw) in split_tiles(M, 256):
                wt = et.tile([128, mw], F32, tag=f"xw{mw}", name=f"xw{tagp}")
                pt = et.tile([128, mw], F32, tag=f"xq{mw}", name=f"xq{tagp}")
                nc.sync.dma_start(out=wt[:pw], in_=wr_[c0:c0 + pw,
                                                       m0:m0 + mw])
                nc.sync.dma_start(out=pt[:pw], in_=pr_[c0:c0 + pw,
                                                       m0:m0 + mw])
                E = ep.tile([128, K * mw], F32, tag=f"E{mw}",
                            name=f"E{tagp}")
                for k in range(K):
                    d = et.tile([128, mw], F32, tag=f"xd{mw}",
                                name=f"xd{tagp}")
                    nc.vector.tensor_scalar(d[:pw], pt[:pw],
                                            float(k - K // 2),
                                            None, OP.subtract)
                    sq = et.tile([128, mw], F32, tag=f"xs{mw}",
                                 name=f"xs{tagp}")
                    nc.vector.tensor_mul(sq[:pw], d[:pw], d[:pw])
                    # exp(-(d/sig)^2/2) = exp(sq * -1/(2 sig^2))
                    nc.scalar.activation(out=E[:pw, k * mw:(k + 1) * mw],
                                         in_=sq[:pw], func=AF.Exp,
                                         scale=float(-0.5 / (c.SIG * c.SIG)))
                esum = et.tile([128, mw], F32, tag=f"xe{mw}",
                               name=f"xe{tagp}")
                nc.vector.reduce_sum(
                    out=esum[:pw],
                    in_=E[:pw].rearrange("p (k m) -> p m k", m=mw),
                    axis=mybir.AxisListType.X)
                nc.vector.tensor_scalar_add(esum[:pw], esum[:pw], 1e-7)
                rinv = et.tile([128, mw], F32, tag=f"xr{mw}",
                               name=f"xr{tagp}")
                nc.vector.reciprocal(rinv[:pw], esum[:pw])
                wn = et.tile([128, mw], F32, tag=f"xn{mw}", name=f"xn{tagp}")
                nc.vector.tensor_mul(wn[:pw], wt[:pw], rinv[:pw])
                for k in range(K):
                    sl = E[:pw, k * mw:(k + 1) * mw]
                    nc.vector.tensor_mul(sl, sl, wn[:pw])
                E3v = E[:pw].rearrange("p (k m) -> p k m", m=mw)
                if split is None:
                    nc.sync.dma_start(
                        out=dst[:, c0:c0 + pw, m0:m0 + mw].rearrange(
                            "k p m -> p k m"),
                        in_=E3v)
                else:
                    Wh = ep.tile([128, K * mw], BF16, tag=f"Wh{mw}",
                                 name=f"Wh{tagp}")
                    nc.vector.tensor_copy(Wh[:pw], E[:pw])
                    h3v = Wh[:pw].rearrange("p (k m) -> p k m", m=mw)
                    if split == 1:
                        nc.sync.dma_start(
                            out=dst[:, c0:c0 + pw, m0:m0 + mw].rearrange(
                                "k p m -> p k m"),
                            in_=h3v)
                    else:
                        nc.sync.dma_start(
                            out=dst[0][:, c0:c0 + pw, m0:m0 + mw].rearrange(
                                "k p m -> p k m"),
                            in_=h3v)
                        Wl = ep.tile([128, K * mw], BF16, tag=f"Wlo{mw}",
                                     name=f"Wl{tagp}")
                        nc.vector.tensor_sub(Wl[:pw], E[:pw], Wh[:pw])
                        nc.sync.dma_start(
                            out=dst[1][:, c0:c0 + pw, m0:m0 + mw].rearrange(
                                "k p m -> p k m"),
                            in_=Wl[:pw].rearrange("p (k m) -> p k m", m=mw))

        # layer-0 weights first: conv1 depends on them
        for ci, (c0, pw) in enumerate(cts_J):
            expand_ct(w0r.ap(), p0r.ap(), c0, pw, H, w0x, None, f"0{ci}")
        # layer-1 / readout expansion emitted interleaved with conv1 below
        pend = [("1", ci, c0, pw) for ci, (c0, pw) in enumerate(cts_H)]
        pend += [("r", ci, c0, pw) for ci, (c0, pw) in enumerate(cts_H)]

        def emit_pending(n):
            while pend and n > 0:
                lay, ci, c0, pw = pend.pop(0)
                if lay == "1":
                    expand_ct(w1r.ap(), p1r.ap(), c0, pw, H, w1x, 2, f"1{ci}")
                else:
                    expand_ct(wrr.ap(), prr.ap(), c0, pw, O, wrx, 1, f"r{ci}")
                n -= 1

        # =============== generic delay-conv from DRAM src =================
        def conv_gen(src, Cin, sdt, wx, planes, wdt, M, tts, chunk_tt,
                     yd=None, sumt=None, sqt=None, nslots=0, y3=None,
                     tag="", hook=None):
            """y[m, t] = sum_{c,k,s} Wx[s,k,c,m]^T src[c, t+k] (padded src)."""
            cts = split_tiles(Cin, 128)
            MT = (M + 127) // 128
            tchunks = split_tiles(len(tts), chunk_tt)
            with ExitStack() as pc:
                psum = pc.enter_context(tc.tile_pool(name=f"psum{tag}",
                                                     bufs=8, space="PSUM"))
                swp = pc.enter_context(tc.tile_pool(name=f"swin{tag}", bufs=2))
                wp = pc.enter_context(tc.tile_pool(name=f"w{tag}", bufs=2))
                sg = pc.enter_context(tc.tile_pool(name=f"stg{tag}", bufs=3))
                for (tci, ntt) in tchunks:
                    tt_group = tts[tci:tci + ntt]
                    w0_ = tt_group[0][0]
                    last_t0, last_nt = tt_group[-1]
                    winlen = (last_t0 + last_nt - 1 + K - 1) - w0_ + 1
                    swin = []
                    for ci2, (cc0, pw) in enumerate(cts):
                        sw = swp.tile([128, winlen * B], sdt, tag=f"sw{ci2}",
                                      name=f"sw{tag}")
                        nc.sync.dma_start(
                            out=sw[:pw].rearrange("p (t b) -> p t b", b=B),
                            in_=src[cc0:cc0 + pw, w0_:w0_ + winlen, :])
                        swin.append(sw)
                    for ht in range(MT):
                        m0 = ht * 128
                        mtw = min(128, M - m0)
                        pss = [psum.tile([128, nt * B], F32, tag="cvps",
                                         name=f"ps{tag}")
                               for (t0, nt) in tt_group]
                        n_acc = len(cts) * K * planes
                        mi = 0
                        for ci2, (cc0, pw) in enumerate(cts):
                            wt = wp.tile([128, planes * K * mtw], wdt,
                                         tag="wt", name=f"wt{tag}")
                            for s in range(planes):
                                wsl = wt[:pw, s * K * mtw:(s + 1) * K * mtw]
                                wsrc = wx if planes == 1 else wx[s]
                                nc.sync.dma_start(
                                    out=wsl.rearrange("p (k m) -> p k m",
                                                      m=mtw),
                                    in_=wsrc[:, cc0:cc0 + pw,
                                             m0:m0 + mtw].rearrange(
                                                 "k p m -> p k m"))
                            for s in range(planes):
                                for kk in range(K):
                                    lhsT = wt[:pw, (s * K + kk) * mtw:
                                              (s * K + kk + 1) * mtw]
                                    st = (mi == 0)
                                    sp_ = (mi == n_acc - 1)
                                    for ti, (t0, nt) in enumerate(tt_group):
                                        off = (t0 - w0_ + kk) * B
                                        nc.tensor.matmul(
                                            pss[ti][:mtw], lhsT=lhsT,
                                            rhs=swin[ci2][:pw,
                                                          off:off + nt * B],
                                            start=st, stop=sp_)
                                    mi += 1
                        for ti, (t0, nt) in enumerate(tt_group):
                            stg = sg.tile([128, nt * B], F32, tag="stg",
                                          name=f"stg{tag}")
                            if sumt is not None:
                                slot = ht * nslots + tci + ti
                                nc.scalar.activation(
                                    out=stg[:mtw], in_=pss[ti][:mtw],
                                    func=AF.Copy,
                                    accum_out=sumt[:, slot:slot + 1])
                                sqg = sg.tile([128, nt * B], F32, tag="sqg",
                                              name=f"sqg{tag}")
                                nc.scalar.activation(
                                    out=sqg[:mtw], in_=pss[ti][:mtw],
                                    func=AF.Square,
                                    accum_out=sqt[:, slot:slot + 1])
                            else:
                                nc.scalar.activation(out=stg[:mtw],
                                                     in_=pss[ti][:mtw],
                                                     func=AF.Copy)
                            if yd is not None:
                                nc.sync.dma_start(
                                    out=yd[m0:m0 + mtw, t0:t0 + nt, :],
                                    in_=stg[:mtw].rearrange(
                                        "p (t b) -> p t b", b=B))
                            else:  # readout: y3 is [T3, B, O], scatter store
                                nc.sync.dma_start(
                                    out=y3[t0:t0 + nt].rearrange(
                                        "t b o -> o t b"),
                                    in_=stg[:mtw].rearrange(
                                        "p (t b) -> p t b", b=B))
                    if hook is not None:
                        hook(1)

        # =============== BN stats: allreduce + affine ===============
        def bn_affine(sumt, sqt, nslots, N, gam, bet, cci, cco, A, Cb, tagp):
            with ExitStack() as pb:
                sp = pb.enter_context(tc.tile_pool(name=f"bn{tagp}", bufs=1))
                ccs = sp.tile([128, 2 * HT], F32, name=f"ccs{tagp}")
                nc.vector.reduce_sum(
                    out=ccs[:, 0:HT],
                    in_=sumt.rearrange("p (h s) -> p h s", s=nslots),
                    axis=mybir.AxisListType.X)
                nc.vector.reduce_sum(
                    out=ccs[:, HT:2 * HT],
                    in_=sqt.rearrange("p (h s) -> p h s", s=nslots),
                    axis=mybir.AxisListType.X)
                nc.sync.dma_start(out=cci, in_=ccs)
                nc.gpsimd.collective_compute(
                    "AllReduce", OP.add,
                    replica_groups=[list(range(c.n_cores))],
                    ins=[cci], outs=[cco])
                gs = sp.tile([128, 2 * HT], F32, name=f"gs{tagp}")
                nc.sync.dma_start(out=gs, in_=cco)
                rN = float(1.0 / N)
                mu = sp.tile([128, HT], F32, name=f"mu{tagp}")
                nc.vector.tensor_scalar(mu, gs[:, 0:HT], rN, None, OP.mult)
                ex2 = sp.tile([128, HT], F32, name=f"ex2{tagp}")
                nc.vector.tensor_scalar(ex2, gs[:, HT:2 * HT], rN, None,
                                        OP.mult)
                var = sp.tile([128, HT], F32, name=f"var{tagp}")
                # var = ex2 - mu*mu ; then + eps
                nc.vector.scalar_tensor_tensor(out=var, in0=mu, scalar=1.0,
                                               in1=mu, op0=OP.mult,
                                               op1=OP.mult)
                nc.vector.tensor_sub(var, ex2, var)
                nc.vector.tensor_scalar_add(var, var, float(c.EPS))
                sv = sp.tile([128, HT], F32, name=f"sv{tagp}")
                nc.scalar.activation(out=sv, in_=var, func=AF.Sqrt)
                # one Newton step: s' = 0.5*(s + v/s)  (ACT sqrt is ~3e-6)
                rs0 = sp.tile([128, HT], F32, name=f"rs0{tagp}")
                nc.vector.reciprocal(rs0, sv)
                t1 = sp.tile([128, HT], F32, name=f"t1{tagp}")
                nc.vector.tensor_mul(t1, var, rs0)
                nc.vector.tensor_add(sv, sv, t1)
                nc.vector.tensor_scalar(sv, sv, 0.5, None, OP.mult)
                rsv = sp.tile([128, HT], F32, name=f"rsv{tagp}")
                nc.vector.reciprocal(rsv, sv)
                nc.vector.tensor_mul(A, gam, rsv)
                # Cbias = bet - mu*A, broadcast over batch
                cb1 = sp.tile([128, HT], F32, name=f"cb1{tagp}")
                nc.vector.tensor_mul(cb1, mu, A)
                nc.vector.tensor_sub(cb1, bet, cb1)
                nc.vector.tensor_copy(
                    Cb.rearrange("p (h b) -> p h b", b=B), bc(cb1, 2, B))

        # =============== LIF layer ===============
        def lif_layer(yd, sd, A, Cb, T, tag):
            with ExitStack() as pl:
                lp = pl.enter_context(tc.tile_pool(name=f"lif{tag}", bufs=2))
                up = pl.enter_context(tc.tile_pool(name=f"lifu{tag}", bufs=1))
                HTB = HT * B
                U = up.tile([128, HTB], F32, name=f"U{tag}")
                nc.vector.memset(U, 0.0)
                for (c0, cn) in split_tiles(T, c.CH):
                    ybufs = []
                    for ht in range(HT):
                        yb = lp.tile([128, cn * B], F32, tag=f"yb{ht}",
                                     name=f"yb{tag}")
                        nc.sync.dma_start(
                            out=yb.rearrange("p (t b) -> p t b", b=B),
                            in_=yd[ht * 128:(ht + 1) * 128, c0:c0 + cn, :])
                        ybufs.append(yb)
                    scn = lp.tile([128, cn * HTB], F32, tag="scn",
                                  name=f"scn{tag}")
                    scn3 = scn.rearrange("p (t x) -> p t x", x=HTB)
                    for ht in range(HT):
                        nc.vector.scalar_tensor_tensor(
                            out=scn3[:, :, ht * B:(ht + 1) * B],
                            in0=ybufs[ht].rearrange("p (t b) -> p t b", b=B),
                            scalar=A[:, ht:ht + 1],
                            in1=bc(Cb[:, ht * B:(ht + 1) * B], 1, cn),
                            op0=OP.mult, op1=OP.add)
                    S = lp.tile([128, cn * HTB], BF16, tag="S", name=f"S{tag}")
                    for t in range(cn):
                        sl = slice(t * HTB, (t + 1) * HTB)
                        ut = lp.tile([128, HTB], F32, tag="ut",
                                     name=f"ut{tag}")
                        nc.vector.scalar_tensor_tensor(
                            out=ut, in0=U, scalar=float(c.BETA),
                            in1=scn[:, sl], op0=OP.mult, op1=OP.add)
                        # spike on ACT (parallel with DVE recurrence):
                        # sign(ut - thresh) in {-1,0,1}, then relu -> {0,1}
                        sg = lp.tile([128, HTB], F32, tag="sg",
                                     name=f"sg{tag}")
                        nc.scalar.activation(out=sg, in_=ut, func=AF.Sign,
                                             bias=negth[:, 0:1])
                        nc.scalar.activation(out=S[:, sl], in_=sg,
                                             func=AF.Relu)
                        nc.vector.scalar_tensor_tensor(
                            out=U, in0=ut, scalar=float(c.THRESH), in1=ut,
                            op0=OP.is_lt, op1=OP.mult)
                    S3 = S.rearrange("p (t h b) -> p t h b", h=HT, b=B)
                    for ht in range(HT):
                        nc.sync.dma_start(
                            out=sd[ht * 128:(ht + 1) * 128,
                                   c.LPAD + c0:c.LPAD + c0 + cn, :],
                            in_=S3[:, :, ht, :])

        # =============== emit the network ===============
        conv_gen(xp.ap(), J, F32, w0x, 1, F32, H, tts1, c.CT1,
                 yd=y1d, sumt=sum1, sqt=sq1, nslots=n1slots, tag="c1",
                 hook=emit_pending)
        emit_pending(len(pend))
        exp_pools.close()

        bn_affine(sum1, sq1, n1slots, c.T1 * c.B_tot, gam0, bet0,
                  cc1i, cc1o, A1, C1b, "1")
        lif_layer(y1d, s1d, A1, C1b, c.T1, "1")
        conv_gen(s1d, H, BF16, w1x, 2, BF16, H, tts2, c.CT2,
                 yd=y2d, sumt=sum2, sqt=sq2, nslots=n2slots, tag="c2")
        bn_affine(sum2, sq2, n2slots, c.T2 * c.B_tot, gam1, bet1,
                  cc2i, cc2o, A2, C2b, "2")
        lif_layer(y2d, s2d, A2, C2b, c.T2, "2")
        conv_gen(s2d, H, BF16, wrx, 1, BF16, O, tts3, c.CT2,
                 y3=y3t, tag="c3")

        # =============== tail: LI via L-matmul, softmax, t-sum ===========
        with ExitStack() as pt:
            psum = pt.enter_context(tc.tile_pool(name="psumt", bufs=6,
                                                 space="PSUM"))
            accp = pt.enter_context(tc.tile_pool(name="accpt", bufs=2,
                                                 space="PSUM"))
            tp = pt.enter_context(tc.tile_pool(name="tail", bufs=1))
            tp2 = pt.enter_context(tc.tile_pool(name="tail2", bufs=2))
            tblocks = split_tiles(c.T3, 128)
            nb = len(tblocks)
            BO = B * O
            halves = split_tiles(BO, 320)
            E3s, Lsb = [], {}
            for i, (tb0, tn) in enumerate(tblocks):
                E3 = tp.tile([128, BO], F32, name=f"E3{i}")
                nc.sync.dma_start(
                    out=E3[:tn].rearrange("p (b o) -> p b o", o=O),
                    in_=y3t[tb0:tb0 + tn])
                E3s.append(E3)
                for jj in range(i, nb):
                    tj0, tjn = tblocks[jj]
                    Lt = tp.tile([128, 128], F32, name=f"L{i}_{jj}")
                    nc.sync.dma_start(out=Lt[:tn, :tjn],
                                      in_=Lm.ap()[tb0:tb0 + tn,
                                                  tj0:tj0 + tjn])
                    Lsb[(i, jj)] = Lt
            ones = tp.tile([128, 1], F32, name="ones")
            nc.vector.memset(ones, 1.0)
            accs = [accp.tile([1, hn], F32, tag=f"acc{hi}", name="acc",
                              bufs=1)
                    for hi, (h0, hn) in enumerate(halves)]
            res = tp.tile([1, BO], F32, name="res")
            for j, (tj0, tjn) in enumerate(tblocks):
                for hi, (h0, hn) in enumerate(halves):
                    ups = psum.tile([128, hn], F32, tag="ups", name="ups")
                    for i in range(j + 1):
                        ti0, tin = tblocks[i]
                        nc.tensor.matmul(
                            ups[:tjn], lhsT=Lsb[(i, j)][:tin, :tjn],
                            rhs=E3s[i][:tin, h0:h0 + hn],
                            start=(i == 0), stop=(i == j))
                    eu = tp2.tile([128, hn], F32, tag="eu", name="eu")
                    nc.scalar.activation(out=eu[:tjn], in_=ups[:tjn],
                                         func=AF.Exp)
                    # per-(t,b) sum over o, then normalize
                    nob = hn // O
                    se = tp2.tile([128, nob], F32, tag="se", name="se")
                    eu3 = eu.rearrange("p (b o) -> p b o", o=O)
                    nc.vector.reduce_sum(out=se[:tjn], in_=eu3[:tjn],
                                         axis=mybir.AxisListType.X)
                    rse = tp2.tile([128, nob], F32, tag="rse", name="rse")
                    nc.vector.reciprocal(rse[:tjn], se[:tjn])
                    pn = tp2.tile([128, hn], F32, tag="pn", name="pn")
                    pn3 = pn.rearrange("p (b o) -> p b o", o=O)
                    nc.vector.tensor_mul(pn3[:tjn], eu3[:tjn],
                                         bc(rse[:tjn], 2, O))
                    nc.tensor.matmul(
                        accs[hi], lhsT=ones[:tjn], rhs=pn[:tjn],
                        start=(j == 0), stop=(j == nb - 1),
                        skip_group_check=True)
            for hi, (h0, hn) in enumerate(halves):
                nc.scalar.copy(out=res[:, h0:h0 + hn], in_=accs[hi])
            nc.sync.dma_start(out=out.ap(), in_=res)

        if c.dbg:
            nc.sync.dma_start(out=d_y1.ap(), in_=y1d)
            nc.sync.dma_start(out=d_s1.ap(), in_=s1d)
            nc.sync.dma_start(out=d_y2.ap(), in_=y2d)
            nc.sync.dma_start(out=d_y3.ap(), in_=y3t)
            nc.sync.dma_start(out=d_w0.ap(), in_=w0x)
            nc.sync.dma_start(out=d_w1.ap(), in_=w1x)

    nc.compile()
    return nc


# ======================= host side =======================

def dcls_np(w, p, K, SIG):
    w = np.asarray(w, np.float32)
    p = np.asarray(p, np.float32)
    idx = np.arange(K, dtype=np.float32)
    d = idx[None, None, :] - np.float32(K // 2) - p[:, :, None]
    t = d / np.float32(SIG)
    g = np.exp(np.float32(-0.5) * t * t).astype(np.float32)
    g = g / (np.sum(g, axis=-1, keepdims=True, dtype=np.float32)
             + np.float32(1e-7))
    return (w[:, :, None] * g).astype(np.float32)


def make_in_maps(cfg: Cfg, x, w0, p0, g0, b0, w1, p1, g1, b1, wr, pr):
    c = cfg

    def chanmat(v):
        return np.ascontiguousarray(
            np.asarray(v, np.float32).reshape(c.HT, 128).T)

    tt = np.arange(c.T3, dtype=np.int64)
    D = tt[None, :] - tt[:, None]
    Lmat = np.where(D >= 0,
                    np.float32(c.BETA) ** np.maximum(D, 0).astype(np.float32),
                    np.float32(0.0)).astype(np.float32)

    shared = {
        "w0r": np.ascontiguousarray(np.asarray(w0, np.float32).T),
        "p0r": np.ascontiguousarray(np.asarray(p0, np.float32).T),
        "w1r": np.ascontiguousarray(np.asarray(w1, np.float32).T),
        "p1r": np.ascontiguousarray(np.asarray(p1, np.float32).T),
        "wrr": np.ascontiguousarray(np.asarray(wr, np.float32).T),
        "prr": np.ascontiguousarray(np.asarray(pr, np.float32).T),
        "Lm": Lmat,
        "g0m": chanmat(g0), "b0m": chanmat(b0),
        "g1m": chanmat(g1), "b1m": chanmat(b1),
    }

    in_maps = []
    x = np.asarray(x, np.float32)
    for ci in range(c.n_cores):
        xs = x[ci * c.B_loc:(ci + 1) * c.B_loc]          # (B_loc, T0, J)
        xpad = np.zeros((c.J, c.T0 + c.PADT, c.B_loc), np.float32)
        xpad[:, c.LPAD:c.LPAD + c.T0, :] = xs.transpose(2, 1, 0)
        m = dict(shared)
        m["xp"] = xpad
        in_maps.append(m)
    return in_maps


_CACHE = {}


def _get_nc(cfg: Cfg):
    key = (cfg.T0, cfg.B_loc, cfg.J, cfg.H, cfg.O, cfg.K, cfg.n_cores,
           cfg.dbg)
    if key not in _CACHE:
        _CACHE[key] = build_kernel(cfg)
    return _CACHE[key]


def run(cfg: Cfg, inputs, trace=False):
    nc = _get_nc(cfg)
    in_maps = make_in_maps(cfg, **inputs)
    res = run_bass_kernel_spmd(nc, in_maps, core_ids=list(range(cfg.n_cores)),
                               trace=trace)
    outs = [res.results[ci]["out"].reshape(cfg.B_loc, cfg.O)
            for ci in range(cfg.n_cores)]
    return np.concatenate(outs, axis=0), res


def kernel(**inputs):
    cfg = Cfg()
    out, _ = run(cfg, inputs)
    return out


# revision 8
# speedup vs baseline: 1.0659x; 1.0659x over previous
"""Trainium2 Bass kernel for nn_DelayLIFSNN.

Architecture (per reference):
  x (B, T0, J) -> delay_conv(w0,p0) -> BN(global batch stats) -> LIF
               -> delay_conv(w1,p1) -> BN -> LIF
               -> delay_conv(wr,pr) -> LI readout -> sum_t softmax_o -> (B, O)

Sharding: data-parallel over batch B across 8 cores (B_loc=32/core);
weights replicated; BN stats all-reduced ((128, 2*HT) f32 = 4KB each).

Wire format: raw transposed weights (w.T, p.T) — the Dcls gaussian-
interpolated delay kernels (K=25 taps) are expanded ON DEVICE
(DVE/ACT: d = p-(k-12); e = exp(-2 d^2); normalize over k; * w),
cutting host->device traffic from ~40MB/core to ~9MB/core.

Matmul precision strategy (spike flips make the output chaotic in the
conv operand precision; measured: tf32-level quantization -> 3.9e-2 rel
err vs the 2e-2 gate, so fp32r is unusable):
  conv1 (x * w0): plain fp32 matmuls (4 cyc/row).
  conv2 (spikes * w1): weights split hi+lo bf16 (2 matmuls, 1 cyc/row
    each); spikes are 0/1 = EXACT in bf16. Combined weight error ~1e-5.
  conv3 (spikes * wr): bf16-hi only (readout has no threshold
    nonlinearity; measured rel err impact nil).

LIF: DVE runs the 2-op recurrence (ut = beta*U + scn; U = (ut<1)*ut);
Pool (gpsimd) computes spikes (ut>=1 -> bf16) and the BN-affine scn
precompute in parallel.

LI readout + softmax tail via PE: u[t,(b,o)] = sum_t' L[t',t] y3[t',b,o]
with L = beta^(t-t') lower-tri Toeplitz (wire input), then exp (no max
subtraction needed: |u| < 20), per-(t,b) normalize, and a ones-matmul
partition-reduce over t. Output [1, B*O].

Activation layouts:
  conv rhs:   [ch_part<=128, t*B + b]  (DRAM src: [C, Tpad, B])
  conv psum:  [out_part 128, t*B + b] per (ht, time-tile)
  y DRAM:     [C, T, B] f32 ; spikes DRAM: [C, T+PADT, B] bf16
  LIF tiles:  [128, t*(HT*B) + ht*B + b]
  y3t DRAM:   [T3, B, O] f32 (scatter-stored by conv3)
"""

import sys
import numpy as np

try:
    import concourse.bass as bass
except ImportError:  # grading env fallback
    sys.path.insert(0, "/opt/trn_rl_repo")
    import concourse.bass as bass

import concourse.mybir as mybir
import concourse.tile as tile
from contextlib import ExitStack
from concourse import bacc
from concourse.bass_utils import run_bass_kernel_spmd

F32 = mybir.dt.float32
BF16 = mybir.dt.bfloat16
AF = mybir.ActivationFunctionType
OP = mybir.AluOpType


class Cfg:
    def __init__(self, T0=300, B_loc=32, J=140, H=512, O=20, K=25, n_cores=8,
                 BETA=0.95, THRESH=1.0, SIG=0.5, EPS=1e-5, NT=16, CH=48,
                 CT1=5, CT2=6, dbg=False):
        self.T0, self.B_loc, self.J, self.H, self.O, self.K = T0, B_loc, J, H, O, K
        self.n_cores = n_cores
        self.BETA, self.THRESH, self.SIG, self.EPS = BETA, THRESH, SIG, EPS
        self.LPAD, self.RPAD = K - 1, (K - 1) // 2
        self.PADT = self.LPAD + self.RPAD                      # 36
        self.T1 = T0 + self.RPAD                               # 312
        self.T2 = self.T1 + self.RPAD                          # 324
        self.T3 = self.T2 + self.RPAD                          # 336
        self.NT = NT                                           # out-steps per matmul tile
        self.CH = CH                                           # LIF chunk steps
        self.CT1 = CT1                                         # conv1 time-tiles per chunk
        self.CT2 = CT2                                         # conv2/3 time-tiles per chunk
        self.HT = (H + 127) // 128                             # h tiles (4)
        self.B_tot = B_loc * n_cores
        self.dbg = dbg
        # packed replicated-weights tensor layout (rows x 512 cols):
        # sharded 1/n_cores per core on the wire, AllGathered on device
        self.PCOLS = max(H, self.T3, 56)
        self.OFF_W0 = 0
        self.OFF_P0 = J
        self.OFF_W1 = 2 * J
        self.OFF_P1 = 2 * J + H
        self.OFF_MISC = 2 * J + 2 * H      # cols 0:O wrr, O:2O prr, 2O+: g/b
        self.OFF_L = 2 * J + 2 * H + H
        r_raw = self.OFF_L + self.T3
        self.PROWS = ((r_raw + n_cores - 1) // n_cores) * n_cores


def split_tiles(total, size):
    out = []
    t = 0
    while t < total:
        n = min(size, total - t)
        out.append((t, n))
        t += n
    return out


def bc(ap, axis, count):
    """Insert a stride-0 (broadcast) axis at position `axis` of an AP."""
    dims = [list(d) for d in ap.ap]
    dims.insert(axis, [0, count])
    return bass.AP(tensor=ap.tensor, offset=ap.offset, ap=dims)


def build_kernel(cfg: Cfg):
    c = cfg
    B, HT, K, H, O, J = c.B_loc, c.HT, c.K, c.H, c.O, c.J
    nc = bacc.Bacc("TRN2", target_bir_lowering=False, debug=False,
                   num_devices=c.n_cores)

    tts1 = split_tiles(c.T1, c.NT)
    tts2 = split_tiles(c.T2, c.NT)
    tts3 = split_tiles(c.T3, c.NT)
    n1slots = len(tts1)
    n2slots = len(tts2)
    cts_J = split_tiles(J, 128)
    cts_H = split_tiles(H, 128)

    # ---- I/O ----
    xp = nc.dram_tensor("xp", [J, c.T0 + c.PADT, B], F32, kind="ExternalInput")
    wpks = nc.dram_tensor("wpks", [c.PROWS // c.n_cores, c.PCOLS], F32,
                          kind="ExternalInput")
    out = nc.dram_tensor("out", [1, B * O], F32, kind="ExternalOutput")
    if c.dbg:
        d_y1 = nc.dram_tensor("d_y1", [H, c.T1, B], F32, kind="ExternalOutput")
        d_s1 = nc.dram_tensor("d_s1", [H, c.T1 + c.PADT, B], BF16,
                              kind="ExternalOutput")
        d_y2 = nc.dram_tensor("d_y2", [H, c.T2, B], F32, kind="ExternalOutput")
        d_y3 = nc.dram_tensor("d_y3", [c.T3, B, O], F32, kind="ExternalOutput")
        d_w0 = nc.dram_tensor("d_w0", [K, J, H], F32, kind="ExternalOutput")
        d_w1 = nc.dram_tensor("d_w1", [2, K, H, H], BF16, kind="ExternalOutput")

    with tile.TileContext(nc) as tc, ExitStack() as ctx:
        dram = ctx.enter_context(tc.tile_pool(name="dram", bufs=1, space="DRAM"))
        w0x = dram.tile([K, J, H], F32, name="w0x")
        w1x = dram.tile([2, K, H, H], BF16, name="w1x")
        wrx = dram.tile([K, H, O], BF16, name="wrx")
        y1d = dram.tile([H, c.T1, B], F32, name="y1d")
        s1d = dram.tile([H, c.T1 + c.PADT, B], BF16, name="s1d")
        y2d = dram.tile([H, c.T2, B], F32, name="y2d")
        s2d = dram.tile([H, c.T2 + c.PADT, B], BF16, name="s2d")
        y3t = dram.tile([c.T3, B, O], F32, name="y3t")
        cc_space = "Shared" if c.n_cores > 4 else "Local"
        wpkf = dram.tile([c.PROWS, c.PCOLS], F32, name="wpkf",
                         addr_space=cc_space)
        cc1i = dram.tile([128, 2 * HT], F32, name="cc1i")
        cc1o = dram.tile([128, 2 * HT], F32, name="cc1o", addr_space=cc_space)
        cc2i = dram.tile([128, 2 * HT], F32, name="cc2i")
        cc2o = dram.tile([128, 2 * HT], F32, name="cc2o", addr_space=cc_space)

        wpki = dram.tile([c.PROWS // c.n_cores, c.PCOLS], F32, name="wpki")
        nc.sync.dma_start(out=wpki, in_=wpks.ap())
        nc.gpsimd.collective_compute(
            "AllGather", OP.bypass,
            replica_groups=[list(range(c.n_cores))],
            ins=[wpki], outs=[wpkf])
        w0r = wpkf[c.OFF_W0:c.OFF_W0 + J, 0:H]
        p0r = wpkf[c.OFF_P0:c.OFF_P0 + J, 0:H]
        w1r = wpkf[c.OFF_W1:c.OFF_W1 + H, 0:H]
        p1r = wpkf[c.OFF_P1:c.OFF_P1 + H, 0:H]
        wrr = wpkf[c.OFF_MISC:c.OFF_MISC + H, 0:O]
        prr = wpkf[c.OFF_MISC:c.OFF_MISC + H, O:2 * O]
        gbm = wpkf[c.OFF_MISC:c.OFF_MISC + 128, 2 * O:2 * O + 4 * HT]
        Lm = wpkf[c.OFF_L:c.OFF_L + c.T3, 0:c.T3]

        glob = ctx.enter_context(tc.tile_pool(name="glob", bufs=1))

        # persistent small tiles
        sum1 = glob.tile([128, HT * n1slots], F32, name="sum1")
        sq1 = glob.tile([128, HT * n1slots], F32, name="sq1")
        sum2 = glob.tile([128, HT * n2slots], F32, name="sum2")
        sq2 = glob.tile([128, HT * n2slots], F32, name="sq2")
        gam0 = glob.tile([128, HT], F32, name="gam0")
        bet0 = glob.tile([128, HT], F32, name="bet0")
        gam1 = glob.tile([128, HT], F32, name="gam1")
        bet1 = glob.tile([128, HT], F32, name="bet1")
        nc.sync.dma_start(out=gam0, in_=gbm[:, 0 * HT:1 * HT])
        nc.sync.dma_start(out=bet0, in_=gbm[:, 1 * HT:2 * HT])
        nc.sync.dma_start(out=gam1, in_=gbm[:, 2 * HT:3 * HT])
        nc.sync.dma_start(out=bet1, in_=gbm[:, 3 * HT:4 * HT])
        A1 = glob.tile([128, HT], F32, name="A1")
        C1b = glob.tile([128, HT * B], F32, name="C1b")
        A2 = glob.tile([128, HT], F32, name="A2")
        C2b = glob.tile([128, HT * B], F32, name="C2b")
        zpad = glob.tile([128, c.LPAD * B], BF16, name="zpad")
        nc.vector.memset(zpad, 0.0)
        negth = glob.tile([128, 1], F32, name="negth")
        nc.vector.memset(negth, float(-cfg.THRESH))

        # zero the pad regions of the spike dram buffers
        for sd, T in ((s1d, c.T1), (s2d, c.T2)):
            for ct, (h0, pw) in enumerate(cts_H):
                nc.sync.dma_start(
                    out=sd[h0:h0 + pw, 0:c.LPAD, :],
                    in_=zpad.rearrange("p (t b) -> p t b", b=B)[:pw])
                nc.sync.dma_start(
                    out=sd[h0:h0 + pw, T + c.LPAD:T + c.PADT, :],
                    in_=zpad.rearrange("p (t b) -> p t b", b=B)[:pw, :c.RPAD, :])

        # =============== weight expansion (device-side Dcls gauss) ========
        exp_pools = ExitStack()
        ep = exp_pools.enter_context(tc.tile_pool(name="exp", bufs=1))
        et = exp_pools.enter_context(tc.tile_pool(name="expt", bufs=2))

        def expand_ct(wr_, pr_, c0, pw, M, dst, split, tagp):
            """Expand one Cin tile: dst[k, c0:c0+pw, :] = w*gauss_norm(k,p).

            Processes M in chunks of <=256 columns to bound SBUF.
            split=None: dst is f32 [K, C, M].
            split=2: dst is bf16 [2, K, C, M] (hi, lo planes).
            split=1: dst is bf16 [K, C, M] (hi only).
            """
            for (m0, mw) in split_tiles(M, 256):
                wt = et.tile([128, mw], F32, tag=f"xw{mw}", name=f"xw{tagp}")
                pt = et.tile([128, mw], F32, tag=f"xq{mw}", name=f"xq{tagp}")
                nc.sync.dma_start(out=wt[:pw], in_=wr_[c0:c0 + pw,
                                                       m0:m0 + mw])
                nc.sync.dma_start(out=pt[:pw], in_=pr_[c0:c0 + pw,
                                                       m0:m0 + mw])
                E = ep.tile([128, K * mw], F32, tag=f"E{mw}",
                            name=f"E{tagp}")
                for k in range(K):
                    d = et.tile([128, mw], F32, tag=f"xd{mw}",
                                name=f"xd{tagp}")
                    nc.vector.tensor_scalar(d[:pw], pt[:pw],
                                            float(k - K // 2),
                                            None, OP.subtract)
                    sq = et.tile([128, mw], F32, tag=f"xs{mw}",
                                 name=f"xs{tagp}")
                    nc.vector.tensor_mul(sq[:pw], d[:pw], d[:pw])
                    # exp(-(d/sig)^2/2) = exp(sq * -1/(2 sig^2))
                    nc.scalar.activation(out=E[:pw, k * mw:(k + 1) * mw],
                                         in_=sq[:pw], func=AF.Exp,
                                         scale=float(-0.5 / (c.SIG * c.SIG)))
                esum = et.tile([128, mw], F32, tag=f"xe{mw}",
                               name=f"xe{tagp}")
                nc.vector.reduce_sum(
                    out=esum[:pw],
                    in_=E[:pw].rearrange("p (k m) -> p m k", m=mw),
                    axis=mybir.AxisListType.X)
                nc.vector.tensor_scalar_add(esum[:pw], esum[:pw], 1e-7)
                rinv = et.tile([128, mw], F32, tag=f"xr{mw}",
                               name=f"xr{tagp}")
                nc.vector.reciprocal(rinv[:pw], esum[:pw])
                wn = et.tile([128, mw], F32, tag=f"xn{mw}", name=f"xn{tagp}")
                nc.vector.tensor_mul(wn[:pw], wt[:pw], rinv[:pw])
                for k in range(K):
                    sl = E[:pw, k * mw:(k + 1) * mw]
                    nc.vector.tensor_mul(sl, sl, wn[:pw])
                E3v = E[:pw].rearrange("p (k m) -> p k m", m=mw)
                if split is None:
                    nc.sync.dma_start(
                        out=dst[:, c0:c0 + pw, m0:m0 + mw].rearrange(
                            "k p m -> p k m"),
                        in_=E3v)
                else:
                    Wh = ep.tile([128, K * mw], BF16, tag=f"Wh{mw}",
                                 name=f"Wh{tagp}")
                    nc.vector.tensor_copy(Wh[:pw], E[:pw])
                    h3v = Wh[:pw].rearrange("p (k m) -> p k m", m=mw)
                    if split == 1:
                        nc.sync.dma_start(
                            out=dst[:, c0:c0 + pw, m0:m0 + mw].rearrange(
                                "k p m -> p k m"),
                            in_=h3v)
                    else:
                        nc.sync.dma_start(
                            out=dst[0][:, c0:c0 + pw, m0:m0 + mw].rearrange(
                                "k p m -> p k m"),
                            in_=h3v)
                        Wl = ep.tile([128, K * mw], BF16, tag=f"Wlo{mw}",
                                     name=f"Wl{tagp}")
                        nc.vector.tensor_sub(Wl[:pw], E[:pw], Wh[:pw])
                        nc.sync.dma_start(
                            out=dst[1][:, c0:c0 + pw, m0:m0 + mw].rearrange(
                                "k p m -> p k m"),
                            in_=Wl[:pw].rearrange("p (k m) -> p k m", m=mw))

        # layer-0 weights first: conv1 depends on them
        for ci, (c0, pw) in enumerate(cts_J):
            expand_ct(w0r, p0r, c0, pw, H, w0x, None, f"0{ci}")
        # layer-1 / readout expansion emitted interleaved with conv1 below
        pend = [("1", ci, c0, pw) for ci, (c0, pw) in enumerate(cts_H)]
        pend += [("r", ci, c0, pw) for ci, (c0, pw) in enumerate(cts_H)]

        def emit_pending(n):
            while pend and n > 0:
                lay, ci, c0, pw = pend.pop(0)
                if lay == "1":
                    expand_ct(w1r, p1r, c0, pw, H, w1x, 2, f"1{ci}")
                else:
                    expand_ct(wrr, prr, c0, pw, O, wrx, 1, f"r{ci}")
                n -= 1

        # =============== generic delay-conv from DRAM src =================
        def conv_gen(src, Cin, sdt, wx, planes, wdt, M, tts, chunk_tt,
                     yd=None, sumt=None, sqt=None, nslots=0, y3=None,
                     tag="", hook=None):
            """y[m, t] = sum_{c,k,s} Wx[s,k,c,m]^T src[c, t+k] (padded src)."""
            cts = split_tiles(Cin, 128)
            MT = (M + 127) // 128
            tchunks = split_tiles(len(tts), chunk_tt)
            with ExitStack() as pc:
                psum = pc.enter_context(tc.tile_pool(name=f"psum{tag}",
                                                     bufs=8, space="PSUM"))
                swp = pc.enter_context(tc.tile_pool(name=f"swin{tag}", bufs=2))
                wp = pc.enter_context(tc.tile_pool(name=f"w{tag}", bufs=2))
                sg = pc.enter_context(tc.tile_pool(name=f"stg{tag}", bufs=3))
                for (tci, ntt) in tchunks:
                    tt_group = tts[tci:tci + ntt]
                    w0_ = tt_group[0][0]
                    last_t0, last_nt = tt_group[-1]
                    winlen = (last_t0 + last_nt - 1 + K - 1) - w0_ + 1
                    swin = []
                    for ci2, (cc0, pw) in enumerate(cts):
                        sw = swp.tile([128, winlen * B], sdt, tag=f"sw{ci2}",
                                      name=f"sw{tag}")
                        nc.sync.dma_start(
                            out=sw[:pw].rearrange("p (t b) -> p t b", b=B),
                            in_=src[cc0:cc0 + pw, w0_:w0_ + winlen, :])
                        swin.append(sw)
                    for ht in range(MT):
                        m0 = ht * 128
                        mtw = min(128, M - m0)
                        pss = [psum.tile([128, nt * B], F32, tag="cvps",
                                         name=f"ps{tag}")
                               for (t0, nt) in tt_group]
                        n_acc = len(cts) * K * planes
                        mi = 0
                        for ci2, (cc0, pw) in enumerate(cts):
                            wt = wp.tile([128, planes * K * mtw], wdt,
                                         tag="wt", name=f"wt{tag}")
                            for s in range(planes):
                                wsl = wt[:pw, s * K * mtw:(s + 1) * K * mtw]
                                wsrc = wx if planes == 1 else wx[s]
                                nc.sync.dma_start(
                                    out=wsl.rearrange("p (k m) -> p k m",
                                                      m=mtw),
                                    in_=wsrc[:, cc0:cc0 + pw,
                                             m0:m0 + mtw].rearrange(
                                                 "k p m -> p k m"))
                            for s in range(planes):
                                for kk in range(K):
                                    lhsT = wt[:pw, (s * K + kk) * mtw:
                                              (s * K + kk + 1) * mtw]
                                    st = (mi == 0)
                                    sp_ = (mi == n_acc - 1)
                                    for ti, (t0, nt) in enumerate(tt_group):
                                        off = (t0 - w0_ + kk) * B
                                        nc.tensor.matmul(
                                            pss[ti][:mtw], lhsT=lhsT,
                                            rhs=swin[ci2][:pw,
                                                          off:off + nt * B],
                                            start=st, stop=sp_)
                                    mi += 1
                        for ti, (t0, nt) in enumerate(tt_group):
                            stg = sg.tile([128, nt * B], F32, tag="stg",
                                          name=f"stg{tag}")
                            if sumt is not None:
                                slot = ht * nslots + tci + ti
                                nc.scalar.activation(
                                    out=stg[:mtw], in_=pss[ti][:mtw],
                                    func=AF.Copy,
                                    accum_out=sumt[:, slot:slot + 1])
                                sqg = sg.tile([128, nt * B], F32, tag="sqg",
                                              name=f"sqg{tag}")
                                nc.scalar.activation(
                                    out=sqg[:mtw], in_=pss[ti][:mtw],
                                    func=AF.Square,
                                    accum_out=sqt[:, slot:slot + 1])
                            else:
                                nc.scalar.activation(out=stg[:mtw],
                                                     in_=pss[ti][:mtw],
                                                     func=AF.Copy)
                            if yd is not None:
                                nc.sync.dma_start(
                                    out=yd[m0:m0 + mtw, t0:t0 + nt, :],
                                    in_=stg[:mtw].rearrange(
                                        "p (t b) -> p t b", b=B))
                            else:  # readout: y3 is [T3, B, O], scatter store
                                nc.sync.dma_start(
                                    out=y3[t0:t0 + nt].rearrange(
                                        "t b o -> o t b"),
                                    in_=stg[:mtw].rearrange(
                                        "p (t b) -> p t b", b=B))
                    if hook is not None:
                        hook(1)

        # =============== BN stats: allreduce + affine ===============
        def bn_affine(sumt, sqt, nslots, N, gam, bet, cci, cco, A, Cb, tagp):
            with ExitStack() as pb:
                sp = pb.enter_context(tc.tile_pool(name=f"bn{tagp}", bufs=1))
                ccs = sp.tile([128, 2 * HT], F32, name=f"ccs{tagp}")
                nc.vector.reduce_sum(
                    out=ccs[:, 0:HT],
                    in_=sumt.rearrange("p (h s) -> p h s", s=nslots),
                    axis=mybir.AxisListType.X)
                nc.vector.reduce_sum(
                    out=ccs[:, HT:2 * HT],
                    in_=sqt.rearrange("p (h s) -> p h s", s=nslots),
                    axis=mybir.AxisListType.X)
                nc.sync.dma_start(out=cci, in_=ccs)
                nc.gpsimd.collective_compute(
                    "AllReduce", OP.add,
                    replica_groups=[list(range(c.n_cores))],
                    ins=[cci], outs=[cco])
                gs = sp.tile([128, 2 * HT], F32, name=f"gs{tagp}")
                nc.sync.dma_start(out=gs, in_=cco)
                rN = float(1.0 / N)
                mu = sp.tile([128, HT], F32, name=f"mu{tagp}")
                nc.vector.tensor_scalar(mu, gs[:, 0:HT], rN, None, OP.mult)
                ex2 = sp.tile([128, HT], F32, name=f"ex2{tagp}")
                nc.vector.tensor_scalar(ex2, gs[:, HT:2 * HT], rN, None,
                                        OP.mult)
                var = sp.tile([128, HT], F32, name=f"var{tagp}")
                # var = ex2 - mu*mu ; then + eps
                nc.vector.scalar_tensor_tensor(out=var, in0=mu, scalar=1.0,
                                               in1=mu, op0=OP.mult,
                                               op1=OP.mult)
                nc.vector.tensor_sub(var, ex2, var)
                nc.vector.tensor_scalar_add(var, var, float(c.EPS))
                sv = sp.tile([128, HT], F32, name=f"sv{tagp}")
                nc.scalar.activation(out=sv, in_=var, func=AF.Sqrt)
                # one Newton step: s' = 0.5*(s + v/s)  (ACT sqrt is ~3e-6)
                rs0 = sp.tile([128, HT], F32, name=f"rs0{tagp}")
                nc.vector.reciprocal(rs0, sv)
                t1 = sp.tile([128, HT], F32, name=f"t1{tagp}")
                nc.vector.tensor_mul(t1, var, rs0)
                nc.vector.tensor_add(sv, sv, t1)
                nc.vector.tensor_scalar(sv, sv, 0.5, None, OP.mult)
                rsv = sp.tile([128, HT], F32, name=f"rsv{tagp}")
                nc.vector.reciprocal(rsv, sv)
                nc.vector.tensor_mul(A, gam, rsv)
                # Cbias = bet - mu*A, broadcast over batch
                cb1 = sp.tile([128, HT], F32, name=f"cb1{tagp}")
                nc.vector.tensor_mul(cb1, mu, A)
                nc.vector.tensor_sub(cb1, bet, cb1)
                nc.vector.tensor_copy(
                    Cb.rearrange("p (h b) -> p h b", b=B), bc(cb1, 2, B))

        # =============== LIF layer ===============
        def lif_layer(yd, sd, A, Cb, T, tag):
            with ExitStack() as pl:
                lp = pl.enter_context(tc.tile_pool(name=f"lif{tag}", bufs=2))
                up = pl.enter_context(tc.tile_pool(name=f"lifu{tag}", bufs=1))
                HTB = HT * B
                U = up.tile([128, HTB], F32, name=f"U{tag}")
                nc.vector.memset(U, 0.0)
                for (c0, cn) in split_tiles(T, c.CH):
                    ybufs = []
                    for ht in range(HT):
                        yb = lp.tile([128, cn * B], F32, tag=f"yb{ht}",
                                     name=f"yb{tag}")
                        nc.sync.dma_start(
                            out=yb.rearrange("p (t b) -> p t b", b=B),
                            in_=yd[ht * 128:(ht + 1) * 128, c0:c0 + cn, :])
                        ybufs.append(yb)
                    scn = lp.tile([128, cn * HTB], F32, tag="scn",
                                  name=f"scn{tag}")
                    scn3 = scn.rearrange("p (t x) -> p t x", x=HTB)
                    for ht in range(HT):
                        nc.vector.scalar_tensor_tensor(
                            out=scn3[:, :, ht * B:(ht + 1) * B],
                            in0=ybufs[ht].rearrange("p (t b) -> p t b", b=B),
                            scalar=A[:, ht:ht + 1],
                            in1=bc(Cb[:, ht * B:(ht + 1) * B], 1, cn),
                            op0=OP.mult, op1=OP.add)
                    S = lp.tile([128, cn * HTB], BF16, tag="S", name=f"S{tag}")
                    for t in range(cn):
                        sl = slice(t * HTB, (t + 1) * HTB)
                        ut = lp.tile([128, HTB], F32, tag="ut",
                                     name=f"ut{tag}")
                        nc.vector.scalar_tensor_tensor(
                            out=ut, in0=U, scalar=float(c.BETA),
                            in1=scn[:, sl], op0=OP.mult, op1=OP.add)
                        # spike on ACT (parallel with DVE recurrence):
                        # sign(ut - thresh) in {-1,0,1}, then relu -> {0,1}
                        sg = lp.tile([128, HTB], F32, tag="sg",
                                     name=f"sg{tag}")
                        nc.scalar.activation(out=sg, in_=ut, func=AF.Sign,
                                             bias=negth[:, 0:1])
                        nc.scalar.activation(out=S[:, sl], in_=sg,
                                             func=AF.Relu)
                        nc.vector.scalar_tensor_tensor(
                            out=U, in0=ut, scalar=float(c.THRESH), in1=ut,
                            op0=OP.is_lt, op1=OP.mult)
                    S3 = S.rearrange("p (t h b) -> p t h b", h=HT, b=B)
                    for ht in range(HT):
                        nc.sync.dma_start(
                            out=sd[ht * 128:(ht + 1) * 128,
                                   c.LPAD + c0:c.LPAD + c0 + cn, :],
                            in_=S3[:, :, ht, :])

        # =============== emit the network ===============
        conv_gen(xp.ap(), J, F32, w0x, 1, F32, H, tts1, c.CT1,
                 yd=y1d, sumt=sum1, sqt=sq1, nslots=n1slots, tag="c1",
                 hook=emit_pending)
        emit_pending(len(pend))
        exp_pools.close()

        bn_affine(sum1, sq1, n1slots, c.T1 * c.B_tot, gam0, bet0,
                  cc1i, cc1o, A1, C1b, "1")
        lif_layer(y1d, s1d, A1, C1b, c.T1, "1")
        conv_gen(s1d, H, BF16, w1x, 2, BF16, H, tts2, c.CT2,
                 yd=y2d, sumt=sum2, sqt=sq2, nslots=n2slots, tag="c2")
        bn_affine(sum2, sq2, n2slots, c.T2 * c.B_tot, gam1, bet1,
                  cc2i, cc2o, A2, C2b, "2")
        lif_layer(y2d, s2d, A2, C2b, c.T2, "2")
        conv_gen(s2d, H, BF16, wrx, 1, BF16, O, tts3, c.CT2,
                 y3=y3t, tag="c3")

        # =============== tail: LI via L-matmul, softmax, t-sum ===========
        with ExitStack() as pt:
            psum = pt.enter_context(tc.tile_pool(name="psumt", bufs=6,
                                                 space="PSUM"))
            accp = pt.enter_context(tc.tile_pool(name="accpt", bufs=2,
                                                 space="PSUM"))
            tp = pt.enter_context(tc.tile_pool(name="tail", bufs=1))
            tp2 = pt.enter_context(tc.tile_pool(name="tail2", bufs=2))
            tblocks = split_tiles(c.T3, 128)
            nb = len(tblocks)
            BO = B * O
            halves = split_tiles(BO, 320)
            E3s, Lsb = [], {}
            for i, (tb0, tn) in enumerate(tblocks):
                E3 = tp.tile([128, BO], F32, name=f"E3{i}")
                nc.sync.dma_start(
                    out=E3[:tn].rearrange("p (b o) -> p b o", o=O),
                    in_=y3t[tb0:tb0 + tn])
                E3s.append(E3)
                for jj in range(i, nb):
                    tj0, tjn = tblocks[jj]
                    Lt = tp.tile([128, 128], F32, name=f"L{i}_{jj}")
                    nc.sync.dma_start(out=Lt[:tn, :tjn],
                                      in_=Lm[tb0:tb0 + tn, tj0:tj0 + tjn])
                    Lsb[(i, jj)] = Lt
            ones = tp.tile([128, 1], F32, name="ones")
            nc.vector.memset(ones, 1.0)
            accs = [accp.tile([1, hn], F32, tag=f"acc{hi}", name="acc",
                              bufs=1)
                    for hi, (h0, hn) in enumerate(halves)]
            res = tp.tile([1, BO], F32, name="res")
            for j, (tj0, tjn) in enumerate(tblocks):
                for hi, (h0, hn) in enumerate(halves):
                    ups = psum.tile([128, hn], F32, tag="ups", name="ups")
                    for i in range(j + 1):
                        ti0, tin = tblocks[i]
                        nc.tensor.matmul(
                            ups[:tjn], lhsT=Lsb[(i, j)][:tin, :tjn],
                            rhs=E3s[i][:tin, h0:h0 + hn],
                            start=(i == 0), stop=(i == j))
                    eu = tp2.tile([128, hn], F32, tag="eu", name="eu")
                    nc.scalar.activation(out=eu[:tjn], in_=ups[:tjn],
                                         func=AF.Exp)
                    # per-(t,b) sum over o, then normalize
                    nob = hn // O
                    se = tp2.tile([128, nob], F32, tag="se", name="se")
                    eu3 = eu.rearrange("p (b o) -> p b o", o=O)
                    nc.vector.reduce_sum(out=se[:tjn], in_=eu3[:tjn],
                                         axis=mybir.AxisListType.X)
                    rse = tp2.tile([128, nob], F32, tag="rse", name="rse")
                    nc.vector.reciprocal(rse[:tjn], se[:tjn])
                    pn = tp2.tile([128, hn], F32, tag="pn", name="pn")
                    pn3 = pn.rearrange("p (b o) -> p b o", o=O)
                    nc.vector.tensor_mul(pn3[:tjn], eu3[:tjn],
                                         bc(rse[:tjn], 2, O))
                    nc.tensor.matmul(
                        accs[hi], lhsT=ones[:tjn], rhs=pn[:tjn],
                        start=(j == 0), stop=(j == nb - 1),
                        skip_group_check=True)
            for hi, (h0, hn) in enumerate(halves):
                nc.scalar.copy(out=res[:, h0:h0 + hn], in_=accs[hi])
            nc.sync.dma_start(out=out.ap(), in_=res)

        if c.dbg:
            nc.sync.dma_start(out=d_y1.ap(), in_=y1d)
            nc.sync.dma_start(out=d_s1.ap(), in_=s1d)
            nc.sync.dma_start(out=d_y2.ap(), in_=y2d)
            nc.sync.dma_start(out=d_y3.ap(), in_=y3t)
            nc.sync.dma_start(out=d_w0.ap(), in_=w0x)
            nc.sync.dma_start(out=d_w1.ap(), in_=w1x)

    nc.compile()
    return nc


# ======================= host side =======================

def dcls_np(w, p, K, SIG):
    w = np.asarray(w, np.float32)
    p = np.asarray(p, np.float32)
    idx = np.arange(K, dtype=np.float32)
    d = idx[None, None, :] - np.float32(K // 2) - p[:, :, None]
    t = d / np.float32(SIG)
    g = np.exp(np.float32(-0.5) * t * t).astype(np.float32)
    g = g / (np.sum(g, axis=-1, keepdims=True, dtype=np.float32)
             + np.float32(1e-7))
    return (w[:, :, None] * g).astype(np.float32)


def make_in_maps(cfg: Cfg, x, w0, p0, g0, b0, w1, p1, g1, b1, wr, pr):
    c = cfg

    def chanmat(v):
        return np.ascontiguousarray(
            np.asarray(v, np.float32).reshape(c.HT, 128).T)

    tt = np.arange(c.T3, dtype=np.int64)
    D = tt[None, :] - tt[:, None]
    Lmat = np.where(D >= 0,
                    np.float32(c.BETA) ** np.maximum(D, 0).astype(np.float32),
                    np.float32(0.0)).astype(np.float32)

    J, H, O = c.J, c.H, c.O
    wpk = np.zeros((c.PROWS, c.PCOLS), np.float32)
    wpk[c.OFF_W0:c.OFF_W0 + J, :H] = np.asarray(w0, np.float32).T
    wpk[c.OFF_P0:c.OFF_P0 + J, :H] = np.asarray(p0, np.float32).T
    wpk[c.OFF_W1:c.OFF_W1 + H, :H] = np.asarray(w1, np.float32).T
    wpk[c.OFF_P1:c.OFF_P1 + H, :H] = np.asarray(p1, np.float32).T
    wpk[c.OFF_MISC:c.OFF_MISC + H, 0:O] = np.asarray(wr, np.float32).T
    wpk[c.OFF_MISC:c.OFF_MISC + H, O:2 * O] = np.asarray(pr, np.float32).T
    gb = np.concatenate([chanmat(g0), chanmat(b0), chanmat(g1), chanmat(b1)],
                        axis=1)
    wpk[c.OFF_MISC:c.OFF_MISC + 128, 2 * O:2 * O + 4 * c.HT] = gb
    wpk[c.OFF_L:c.OFF_L + c.T3, :c.T3] = Lmat

    rsh = c.PROWS // c.n_cores
    in_maps = []
    x = np.asarray(x, np.float32)
    for ci in range(c.n_cores):
        xs = x[ci * c.B_loc:(ci + 1) * c.B_loc]          # (B_loc, T0, J)
        xpad = np.zeros((c.J, c.T0 + c.PADT, c.B_loc), np.float32)
        xpad[:, c.LPAD:c.LPAD + c.T0, :] = xs.transpose(2, 1, 0)
        in_maps.append({
            "xp": xpad,
            "wpks": np.ascontiguousarray(wpk[ci * rsh:(ci + 1) * rsh]),
        })
    return in_maps


_CACHE = {}


def _get_nc(cfg: Cfg):
    key = (cfg.T0, cfg.B_loc, cfg.J, cfg.H, cfg.O, cfg.K, cfg.n_cores,
           cfg.dbg)
    if key not in _CACHE:
        _CACHE[key] = build_kernel(cfg)
    return _CACHE[key]


def run(cfg: Cfg, inputs, trace=False):
    nc = _get_nc(cfg)
    in_maps = make_in_maps(cfg, **inputs)
    res = run_bass_kernel_spmd(nc, in_maps, core_ids=list(range(cfg.n_cores)),
                               trace=trace)
    outs = [res.results[ci]["out"].reshape(cfg.B_loc, cfg.O)
            for ci in range(cfg.n_cores)]
    return np.concatenate(outs, axis=0), res


def kernel(**inputs):
    cfg = Cfg()
    out, _ = run(cfg, inputs)
    return out
